# revision 28
# baseline (speedup 1.0000x reference)
"""Trainium2 kernel for nn_Loss_HF_86079734546730.

Strategy (8 NeuronCores, SPMD, no collectives):
  - Shard the two [64,3,512,512] inputs spatially over H: core k gets raw
    rows [64k, 64k+64) => shard [64, 3, 64, 512] per tensor (48 MiB/core).
  - DMA: 12 loads of 4 MiB each (32 batches x one channel), SBUF layout
    [128 = (b32 x qq4), 8192 = (16 h-rows x 512 w)] fp32 -> 32 KiB
    contiguous per partition. 32 KiB descriptors run all 16 DMA engines at
    full rate (~334 GB/s measured); 2 KiB descriptors cap at ~100 GB/s.
  - Vertical Haar (DVE): vs/vd = x[h even] +/- x[h odd], w-parity
    deinterleaved on write (contiguous writes, stride-2 reads), bf16.
  - Band build (PE): per (tile, wbc, rb-half): 16 bf16 matmuls; stationary
    = contiguous vs/vd w-parity slice [128, 128wb] (FWL-eligible), moving
    = +/-0.5*PI permutation blocks. The horizontal Haar pass rides PSUM
    accumulation (even-w mm + odd-w mm accumulate); hl and hh share one
    N=256 moving. PI reorders psum cols to (qq, b) so copies have
    contiguous inner runs.
  - PSUM->SBUF: 3 copies (scalar engine) per (tile, wbc, half) cast bf16
    into the band buffer [128 wb, 6176 cols], column g = hb*193 + colIdx,
    hb = half*16 + rb4*4 + qq, colIdx = bt*96 + band*32 + b; hb-major so
    the Gram's stationary/moving operands are contiguous (FWL-eligible).
  - Gram (PE): per (t, c): contract the band buffer over spatial (wb
    partitions x 32 hb column-groups x 2 wb-chunks) into PSUM fp32.
    Symmetric trim: chunk0 = rows 0:128 x cols 0:193, chunk1 = rows
    128:193 x cols 128:193; host mirrors. The ones column makes row/col
    192 the per-band sums, so means/stds reconstruct on host.
  - Host (float64): sum partial Grams over cores, rebuild per-(b,c,band)
    mean/std, expand the normalized-feature Gram algebraically,
    cosine-sim, softmax, KL.
"""

import numpy as np

B, C, H, W = 64, 3, 512, 512
NCORES = 8
HSH = H // NCORES          # 64 raw rows per core
EPS_STD = 1e-5
EPS_COS = 1e-8
EPS_P = 1e-8

BPT = 32                   # batches per raw tile
NT = B // BPT              # 2 raw tiles per (t, c)
NBCOL = 6176               # (192 band cols + 1 ones col) x 32 hb

_CACHE = {}


def _make_w():
    """[128, 512] fp32 moving operands: [P|M] and [P|P], P/M = +/-0.5*PI.

    PI is the (b,qq)->(qq,b) permutation: partition p = b*4 + qq lands in
    psum col qq*32 + b, so the PSUM->SBUF copies get contiguous inner runs.
    lh-even mm uses P (cols 0:128), lh-odd uses M (128:256);
    vd-even mm uses [P|M] (0:256) -> (hl|hh), vd-odd uses [P|P] (256:512).
    """
    pi = np.zeros((128, 128), np.float32)
    for b in range(BPT):
        for qq in range(4):
            pi[b * 4 + qq, qq * BPT + b] = 1.0
    w = np.zeros((128, 512), np.float32)
    w[:, 0:128] = 0.5 * pi
    w[:, 128:256] = -0.5 * pi
    w[:, 256:384] = 0.5 * pi
    w[:, 384:512] = 0.5 * pi
    return w


def _col_batch():
    """band-buffer column g = colIdx*32 + hb; colIdx = bt*96 + band*32 + b
    -> batch index bt*32 + b (band order lh, hl, hh; irrelevant to host)."""
    col_batch = np.zeros(192, np.int64)
    for bt in range(NT):
        for band in range(3):
            for b in range(BPT):
                col_batch[bt * 96 + band * 32 + b] = bt * BPT + b
    return col_batch


def _build_nc():
    import concourse.mybir as mybir
    import concourse.tile as tile
    from concourse import bacc

    f32 = mybir.dt.float32
    bf16 = mybir.dt.bfloat16

    nc = bacc.Bacc()
    za = nc.declare_dram_parameter("za", [B, C, HSH, W], f32, isOutput=False)
    zs = nc.declare_dram_parameter("zs", [B, C, HSH, W], f32, isOutput=False)
    wmat = nc.declare_dram_parameter("wmat", [128, 512], bf16, isOutput=False)
    g0 = nc.declare_dram_parameter("G0", [2, C, 128, 193], f32, isOutput=True)
    g1 = nc.declare_dram_parameter("G1", [2, C, 65, 65], f32, isOutput=True)
    zz = [za, zs]

    with tile.TileContext(nc) as tc:
        with (
            tc.tile_pool(name="wconst", bufs=1) as w_pool,
            tc.tile_pool(name="raw", bufs=2) as raw_pool,
            tc.tile_pool(name="vsd", bufs=2) as vsd_pool,
            tc.tile_pool(name="bands", bufs=2) as band_pool,
            tc.tile_pool(name="stage", bufs=2) as stage_pool,
            tc.tile_pool(name="pband", bufs=2, space="PSUM") as pb_pool,
            tc.tile_pool(name="pgram", bufs=2, space="PSUM") as pg_pool,
        ):
            w_t = w_pool.tile([128, 512], bf16, tag="wmat")
            nc.gpsimd.dma_start(w_t[:], wmat[:])
            wp = w_t[:, 0:128]     # +0.5*I
            wm = w_t[:, 128:256]   # -0.5*I
            wpm = w_t[:, 0:256]    # [P|M] -> (hl|hh) from vd_e
            wpp = w_t[:, 256:512]  # [P|P] -> (hl|hh) from vd_o

            for c in range(C):
                bufs = {}
                for t in range(2):
                    for wbc in range(2):
                        bb = band_pool.tile([128, NBCOL], bf16, tag=f"bb{t}{wbc}")
                        bbh = bb[:].rearrange("p (hb col) -> p hb col", col=193)
                        nc.gpsimd.memset(bbh[:, :, 192], 1.0)
                        bufs[(t, wbc)] = bb

                for t in range(2):
                    for bt in range(NT):
                        raw = raw_pool.tile([128, 8192], f32, tag="raw")
                        # sequential-address DMA: partition p = b*4 + qq
                        # (qq = h-quarter); the band matmuls' permutation
                        # moving 0.5*PI reorders psum cols to qq*32 + b so
                        # the copies get contiguous inner runs for free
                        nc.gpsimd.dma_start(
                            raw[:],
                            zz[t][BPT * bt : BPT * (bt + 1), c].rearrange(
                                "b h w -> b (h w)"
                            ),
                        )
                        # raw col = rb*1024 + hpar*512 + wb*2 + wpar
                        # vs/vd col = rb*512 + wpar*256 + wb (parity split so
                        # band-matmul stationaries are contiguous -> FWL);
                        # DVE writes contiguous, reads stride-2
                        rvw = raw[:].rearrange(
                            "p (rb hpar wb wpar) -> p hpar rb wpar wb",
                            rb=8, hpar=2, wb=256, wpar=2,
                        )
                        vs = vsd_pool.tile([128, 4096], bf16, tag="vs")
                        vd = vsd_pool.tile([128, 4096], bf16, tag="vd")
                        vsv = vs[:].rearrange(
                            "p (rb wpar wb) -> p rb wpar wb", rb=8, wpar=2
                        )
                        vdv = vd[:].rearrange(
                            "p (rb wpar wb) -> p rb wpar wb", rb=8, wpar=2
                        )
                        # vertical pass on DVE only: keeps the gpsimd queue
                        # free to issue the next loads without blocking
                        nc.vector.tensor_add(vsv, rvw[:, 0], rvw[:, 1])
                        nc.vector.tensor_sub(vdv, rvw[:, 0], rvw[:, 1])

                        for wbc in range(2):
                            for half in range(2):
                                # psum: lh at rb4*128 (bank 0), (hl|hh) at
                                # 512 + rb4*256 (banks 1-2); all accumulation
                                # groups contiguous, none crossing a bank
                                pb = pb_pool.tile([128, 1536], f32, tag="pband")
                                for rb4 in range(4):
                                    rb = half * 4 + rb4
                                    sve = vs[:, rb * 512 + 128 * wbc :][:, :128]
                                    svo = vs[
                                        :, rb * 512 + 256 + 128 * wbc :
                                    ][:, :128]
                                    sde = vd[:, rb * 512 + 128 * wbc :][:, :128]
                                    sdo = vd[
                                        :, rb * 512 + 256 + 128 * wbc :
                                    ][:, :128]
                                    o_lh = pb[:, rb4 * 128 : rb4 * 128 + 128]
                                    o_vd = pb[
                                        :, 512 + rb4 * 256 : 512 + rb4 * 256 + 256
                                    ]
                                    # lh = +0.5 vs_e - 0.5 vs_o
                                    nc.tensor.matmul(
                                        o_lh, sve, wp, start=True, stop=False
                                    )
                                    nc.tensor.matmul(
                                        o_lh, svo, wm, start=False, stop=True
                                    )
                                    # (hl|hh) = vd_e^T [P|M] + vd_o^T [P|P]
                                    nc.tensor.matmul(
                                        o_vd, sde, wpm, start=True, stop=False
                                    )
                                    nc.tensor.matmul(
                                        o_vd, sdo, wpp, start=False, stop=True
                                    )
                                # copies per band: psum cols (rb4, qq, b) ->
                                # bb g = hb*193 + colIdx, hb = half*16 +
                                # rb4*4 + qq, colIdx = bt*96 + band*32 + b;
                                # inner dim b contiguous on both sides
                                srcs = [
                                    pb[:, 0:512].rearrange(
                                        "p (rb4 qq b) -> p rb4 qq b",
                                        rb4=4, qq=4, b=BPT,
                                    ),
                                    pb[:, 512:1536].rearrange(
                                        "p (rb4 band2 qq b) -> p band2 rb4 qq b",
                                        rb4=4, band2=2, qq=4, b=BPT,
                                    ),
                                ]
                                bbv = bufs[(t, wbc)][:].rearrange(
                                    "p (h2 rb4 qq col) -> p h2 rb4 qq col",
                                    h2=2, rb4=4, qq=4, col=193,
                                )
                                for band in range(3):
                                    src = (
                                        srcs[0]
                                        if band == 0
                                        else srcs[1][:, band - 1]
                                    )
                                    dst = bbv[
                                        :,
                                        half,
                                        :,
                                        :,
                                        bt * 96 + band * 32 : bt * 96
                                        + band * 32
                                        + 32,
                                    ]
                                    nc.scalar.activation(
                                        dst, src,
                                        mybir.ActivationFunctionType.Copy,
                                    )

                    # gram for this tensor immediately after its bands:
                    # shortens the end-of-kernel tail
                    pg0 = pg_pool.tile([128, 193], f32, tag="pg", name="pg0")
                    for wbc in range(2):
                        bbf = bufs[(t, wbc)][:]
                        for hb in range(32):
                            nc.tensor.matmul(
                                pg0[:, :],
                                bbf[:, hb * 193 : hb * 193 + 128],
                                bbf[:, hb * 193 : hb * 193 + 193],
                                start=(wbc == 0 and hb == 0),
                                stop=(wbc == 1 and hb == 31),
                            )
                    st0 = stage_pool.tile([128, 193], f32, tag="st0")
                    nc.scalar.activation(
                        st0[:], pg0[:], mybir.ActivationFunctionType.Copy
                    )
                    nc.sync.dma_start(g0[t, c], st0[:])
                    pg1 = pg_pool.tile([128, 193], f32, tag="pg", name="pg1")
                    for wbc in range(2):
                        bbf = bufs[(t, wbc)][:]
                        for hb in range(32):
                            nc.tensor.matmul(
                                pg1[:65, 0:65],
                                bbf[:, hb * 193 + 128 : hb * 193 + 193],
                                bbf[:, hb * 193 + 128 : hb * 193 + 193],
                                start=(wbc == 0 and hb == 0),
                                stop=(wbc == 1 and hb == 31),
                            )
                    st1 = stage_pool.tile([128, 65], f32, tag="st1")
                    nc.scalar.activation(
                        st1[:65, :], pg1[:65, 0:65],
                        mybir.ActivationFunctionType.Copy,
                    )
                    nc.sync.dma_start(g1[t, c], st1[:65, :])
    if not nc.is_finalized():
        nc.finalize()
    return nc


def _get_nc():
    if "nc" not in _CACHE:
        _CACHE["nc"] = _build_nc()
    return _CACHE["nc"]


def _in_maps(z_ada, z_sou):
    import ml_dtypes

    wm = _make_w().astype(ml_dtypes.bfloat16)
    maps = []
    for k in range(NCORES):
        sl = slice(HSH * k, HSH * (k + 1))
        maps.append(
            {
                "za": np.ascontiguousarray(z_ada[:, :, sl, :]),
                "zs": np.ascontiguousarray(z_sou[:, :, sl, :]),
                "wmat": wm,
            }
        )
    return maps


def _host_finish(g_parts):
    """g_parts: list of per-core (G0 [2,3,128,193], G1 [2,3,65,65]) fp32."""
    s0 = np.zeros((2, C, 128, 193), np.float64)
    s1 = np.zeros((2, C, 65, 65), np.float64)
    for a0, a1 in g_parts:
        s0 += np.asarray(a0, np.float64)
        s1 += np.asarray(a1, np.float64)

    col_batch = _col_batch()
    S = float(s1[0, 0, 64, 64])

    P = np.zeros((2, B, B), np.float64)
    Bm = np.zeros((192, B), np.float64)
    Bm[np.arange(192), col_batch] = 1.0
    for t in range(2):
        for c in range(C):
            full = np.zeros((193, 193), np.float64)
            full[0:128, :] = s0[t, c]
            full[128:193, 128:193] = s1[t, c]
            full[128:193, 0:128] = s0[t, c][:, 128:193].T
            M = full[:192, :192]
            Tv = full[192, :192]
            mu = Tv / S
            var = (np.diag(M) - Tv * Tv / S) / (S - 1.0)
            sig = np.sqrt(np.maximum(var, 0.0))
            alpha = 1.0 / (3.0 * (sig + EPS_STD))
            Mc = M - np.outer(mu, Tv) - np.outer(Tv, mu) + S * np.outer(mu, mu)
            Ms = (alpha[:, None] * Mc) * alpha[None, :]
            P[t] += Bm.T @ Ms @ Bm

    sims = []
    for t in range(2):
        r = np.sqrt(np.maximum(np.diag(P[t]), 0.0))
        rc = np.maximum(r, EPS_COS)
        sims.append(P[t] / np.outer(rc, rc))

    def softmax_offdiag(sim):
        m = sim.copy()
        np.fill_diagonal(m, -np.inf)
        mx = m.max(axis=1, keepdims=True)
        e = np.exp(m - mx)
        return e / e.sum(axis=1, keepdims=True)

    p_ada = softmax_offdiag(sims[0]) + EPS_P
    p_sou = softmax_offdiag(sims[1]) + EPS_P
    kl = np.sum(p_sou * (np.log(p_sou) - np.log(p_ada))) / B
    return np.float32(kl)


def kernel(z_ada, z_sou):
    from concourse.bass_utils import run_bass_kernel_spmd

    z_ada = np.asarray(z_ada, np.float32)
    z_sou = np.asarray(z_sou, np.float32)
    nc = _get_nc()
    res = run_bass_kernel_spmd(nc, _in_maps(z_ada, z_sou), list(range(NCORES)))
    g_parts = [
        (res.results[k]["G0"], res.results[k]["G1"]) for k in range(NCORES)
    ]
    return _host_finish(g_parts)
